# revision 19
# baseline (speedup 1.0000x reference)
"""Transformer-XL relative multi-head attention, 8-way sharded on Trainium2.

Self-contained harness entry: kernel(**inputs) -> np.ndarray [4, 1024, 1024].

Sharding: core c handles batch b = c//2 and head-half hh = c%2 (8 of 16
heads). Each core computes a partial output (its heads' contribution
through Wo); the host unshard sums the two partials per batch (row-parallel
tensor parallelism for the output projection).

Kernel structure ("transposed-scores" form):
  - projections q/k/v/R (bf16 inputs, batched DMA loads)
  - position matrix M = (qh+v)^T . rh2 per query tile, exp'd on the scalar
    engine, written to a DRAM scratch region with row stride W; the
    circulant shift is realized by reading back with row stride W+1
    through a single DMA-transpose per (quad, key-block), which lands
    exp(bd)^T [key, query] tiles directly in SBUF.
  - content scores computed transposed (ac^T = kh^T-block . qu) so the
    AV matmul needs no attention transpose at all; exp on scalar engine.
  - att = exp(ac)*exp(bd) (one fp16 vector multiply); AV accumulates over
    key blocks with an appended ones-column in vh to produce the softmax
    denominator; normalization applied after AV via a partition-broadcast
    reciprocal.
  - heads h0/h1 of each pair are issued back-to-back so their K=64
    matmuls pack into the two row-halves of the PE array.
"""

import os
import sys

sys.path.insert(0, "/opt/trn_rl_repo")

import numpy as np
import ml_dtypes

import concourse.bass as bass
import concourse.mybir as mybir
from concourse.tile import TileContext, ScopedClock

F32 = mybir.dt.float32
F16 = mybir.dt.float16
BF16 = mybir.dt.bfloat16
AF = mybir.ActivationFunctionType
OP = mybir.AluOpType

S, T, D, DK, P = 1024, 2048, 1024, 64, 128
DH = 512          # head-slice width per core (8 heads)
HC = 8
EXP_BIAS = -7.0   # bias inside the single exp (cancels in softmax)
def _w_of(qi):
    return (qi + 9) * 128 + 127


SLOT = [128 * (_w_of(qi) + 1) for qi in range(8)]   # per-qi scratch slot elems
SLOT_OFF = [[sum(SLOT[4 * Q : 4 * Q + j]) for j in range(4)] for Q in range(2)]
QREG = [sum(SLOT[4 * Q : 4 * Q + 4]) for Q in range(2)]  # per-(h,Q) region


def kq_of(qi):  # valid key count for query tile qi (keys j <= i + 1024)
    return (qi + 9) * P


def w_of(qi):  # position-matrix width for query tile qi
    return kq_of(qi) + 127


def cdiv(a, b):
    return (a + b - 1) // b


def _patched_drain_and_barrier(self, tick_clock, wait_clock):
    # The walrus build in this container caps sync-waits per instruction;
    # Tile's stock tail drain carries one wait per live proc. Emit one SP nop
    # per wait instead, then the drain.
    dummy = mybir.InstNoOp(name="drain-wait-probe", ins=[], outs=[])
    dummy.engine = mybir.EngineType.SP
    wait_clock.add_sem_waits(dummy, ScopedClock({None: tick_clock.global_clock}))
    waits = []
    if dummy.sync_info is not None and dummy.sync_info.on_wait:
        waits = [(w.ant_name, w.wait_value) for w in dummy.sync_info.on_wait]
    assert self.sems is not None
    name2sem = {h.name: h for h in self.sems.allocated().values()}
    for name, val in waits:
        self.nc.sync.nop().wait_op(name2sem[name], val, "sem-ge")
    self.nc.sync.drain()
    self.nc.all_engine_barrier()
    popped = self.nc._tile_sem_poison_stack.pop()
    assert popped is self._sem_poison
    self.nc.clear_and_free_semaphores(list(self.sems.allocated().values()))
    self.nc.all_engine_barrier()


TileContext._drain_and_barrier = _patched_drain_and_barrier


def _split_multi_waits(nc, max_waits=1):
    """Walrus in this container rejects instructions carrying more than a
    couple of sync waits. Hoist extras onto same-engine NoOps just before
    the instruction (sequential on the engine, so semantics unchanged)."""
    for f in nc.m.functions:
        for bb in f.blocks:
            out = []
            changed = False
            for inst in bb.instructions:
                si = inst.sync_info
                if si is not None and si.on_wait and len(si.on_wait) > max_waits:
                    waits = list(si.on_wait)
                    for j, w in enumerate(waits[:-max_waits]):
                        nop = mybir.InstNoOp(
                            name=f"{inst.name}-wsplit{j}", ins=[], outs=[])
                        nop.engine = inst.engine
                        nop.sync_info = mybir.SyncInfo(on_wait=[w], on_update=[])
                        out.append(nop)
                    inst.sync_info = mybir.SyncInfo(
                        on_wait=waits[-max_waits:],
                        on_update=list(si.on_update))
                    changed = True
                out.append(inst)
            if changed:
                bb.instructions = out


def build_nc(split_waits=True):
    nc = bass.Bass(target_bir_lowering=True)

    qT = nc.declare_dram_parameter("qT", [D, S], BF16, isOutput=False)
    kT = nc.declare_dram_parameter("kT", [D, T], BF16, isOutput=False)
    vT = nc.declare_dram_parameter("vT", [D, T], BF16, isOutput=False)
    RT = nc.declare_dram_parameter("RT", [D, T], BF16, isOutput=False)
    Wq = nc.declare_dram_parameter("Wq", [D, DH], BF16, isOutput=False)
    Wk = nc.declare_dram_parameter("Wk", [D, DH], BF16, isOutput=False)
    Wv = nc.declare_dram_parameter("Wv", [D, DH], BF16, isOutput=False)
    Wr = nc.declare_dram_parameter("Wr", [D, DH], BF16, isOutput=False)
    Wo16 = nc.declare_dram_parameter("Wo16", [DH, D], F16, isOutput=False)
    ub = nc.declare_dram_parameter("ub", [P, 4], F32, isOutput=False)
    vb = nc.declare_dram_parameter("vb", [P, 4], F32, isOutput=False)
    atril2 = nc.declare_dram_parameter(
        "atril2", [P, P], mybir.dt.uint8, isOutput=False)
    ident = nc.declare_dram_parameter("ident", [P, P], F16, isOutput=False)
    outp = nc.declare_dram_parameter("out", [S, D], F32, isOutput=True)

    with TileContext(nc) as tc:
        with (
            tc.tile_pool(name="persist", bufs=1) as pp,
            tc.tile_pool(name="consts", bufs=1) as cp,
        ):
            # persistent fp16 tensors (partition = dk of the 2 heads in a pair)
            quT = pp.tile([P, 4 * S], F16)       # (qh+u).T   blocks hp
            qvT = pp.tile([P, 4 * S], F16)       # (qh+v).T
            khT = pp.tile([P, 4 * T], F16)
            rh2T = pp.tile([P, 4 * 3072], F16)
            vh16 = pp.tile([P, 16 * (HC * 65)], F16)  # per key tile: 8 h x (64+1)
            concatT = pp.tile([P, 4 * S], F16)
            WoS = pp.tile([P, 4 * D], F16)

            ub_sb = cp.tile([P, 4], F32)
            vb_sb = cp.tile([P, 4], F32)
            atril2_sb = cp.tile([P, P], mybir.dt.uint8)
            zeros_sb = cp.tile([P, P], F16)
            biasn_sb = cp.tile([P, 1], F32)
            ones_sb = cp.tile([P, DK], F32)
            ident_sb = cp.tile([P, P], F16)
            nc.vector.memset(biasn_sb[:], EXP_BIAS)
            nc.vector.memset(zeros_sb[:], 0.0)
            nc.vector.memset(ones_sb[:], 1.0)

            nc.scalar.dma_start(out=ub_sb[:], in_=ub[:])
            nc.scalar.dma_start(out=vb_sb[:], in_=vb[:])
            nc.scalar.dma_start(out=atril2_sb[:], in_=atril2[:])
            nc.scalar.dma_start(out=ident_sb[:], in_=ident[:])
            # WoS layout [128, dt*1024 + o] <- Wo16[(dt p), o], one DMA
            nc.scalar.dma_start(
                out=WoS[:],
                in_=bass.AP(Wo16, 0, [[D, P], [P * D, 4], [1, D]]),
            )

            # ---------------- projections + attention ----------------
            # attention-side pools are opened first so hp0's M stage can be
            # software-pipelined into the k/v projection chains; psm doubles
            # as the projection chain psum pool (PSUM budget: 2+4+2 = 8)
            with (
                tc.tile_pool(name="att_m", bufs=3) as mp,
                tc.tile_pool(name="dram", bufs=10, space="DRAM") as dp,
                tc.tile_pool(name="ps_m", bufs=2, space="PSUM") as psm,
                tc.tile_pool(name="ps_ac", bufs=4, space="PSUM") as psac,
                tc.tile_pool(name="ps_av", bufs=2, space="PSUM") as psav,
            ):
                mregs = {}

                def m_units_for(hp, split_evac):
                    """hp's M stage as a list of small closures, so it can be
                    software-pipelined into other work's emission.
                    One DRAM region per (h, Q) holding that quad's 4 slots."""
                    mregs[hp] = {(h, Q): dp.tile([QREG[Q]], F16, tag="mreg",
                                                 name="mreg")
                                 for h in range(2) for Q in range(2)}
                    units = []
                    msbs_state = {}

                    for qi in range(8):
                        Qd = qi // 4
                        Wq_ = w_of(qi)
                        nwc = cdiv(Wq_, 512)

                        def mk_wc(qi=qi, Qd=Qd, Wq_=Wq_, wc_=None):
                            def f(wc=wc_):
                                if qi not in msbs_state:
                                    msbs_state[qi] = [
                                        mp.tile([P, 2176], F16, tag="msb",
                                                name="msb") for _ in range(2)]
                                nw = min(512, Wq_ - wc * 512)
                                for h in range(2):
                                    pr = slice(h * DK, (h + 1) * DK)
                                    mps = psm.tile([P, 512], F32, tag="mps",
                                                   name="mps")
                                    nc.tensor.matmul(
                                        mps[:, :nw],
                                        qvT[pr, hp * S + qi * P :
                                            hp * S + (qi + 1) * P],
                                        rh2T[pr, hp * 3072 + qi * P + wc * 512 :
                                             hp * 3072 + qi * P + wc * 512 + nw],
                                        start=True, stop=True,
                                    )
                                    dst = msbs_state[qi][h][
                                        :, wc * 512 : wc * 512 + nw]
                                    if split_evac and (qi + h) % 2 == 0:
                                        nc.scalar.activation(
                                            dst, mps[:, :nw], AF.Copy)
                                    else:
                                        nc.vector.tensor_copy(dst, mps[:, :nw])
                            return f

                        for wc in range(nwc):
                            units.append(mk_wc(wc_=wc))

                        def mk_wr(qi=qi, Qd=Qd, Wq_=Wq_, hp=hp):
                            def f():
                                msbs = msbs_state.pop(qi)
                                base = SLOT_OFF[Qd][qi % 4]
                                for h in range(2):
                                    reg = mregs[hp][(h, Qd)]
                                    nc.gpsimd.dma_start(
                                        out=bass.AP(
                                            reg.tensor, reg.offset + base,
                                            [[Wq_, P], [1, Wq_]]),
                                        in_=msbs[h][:, :Wq_],
                                    )
                            return f

                        units.append(mk_wr())
                    return units

                def load_w(pool, wparam):
                    wsb = pool.tile([P, 8 * DH], BF16, tag="wsb")
                    nc.sync.dma_start(
                        out=wsb[:],
                        in_=bass.AP(wparam, 0, [[DH, P], [P * DH, 8], [1, DH]]),
                    )
                    return wsb

                def load_x(pool, xparam, th):
                    # 8 DMAs alternating the two HWDGE rings so the first
                    # chain can start after one chunk lands
                    xsb = pool.tile([P, 8 * 1024], BF16, tag="xstage")
                    L = xparam.shape[1]
                    for kd in range(8):
                        eng = nc.scalar if kd % 2 else nc.sync
                        eng.dma_start(
                            out=xsb[:, kd * 1024 : (kd + 1) * 1024],
                            in_=bass.AP(xparam, kd * P * L + th * 1024,
                                        [[L, P], [1, 1024]]),
                        )
                    return xsb

                def proj_T(pool, wsb, xparam, ncols, evac, pull=None):
                    # chain-major: 8 back-to-back matmuls per psum chain
                    nth = ncols // 1024
                    for th in range(nth):
                        xsb = load_x(pool, xparam, th)
                        for dot in range(4):
                            for tc2 in range(2):
                                ps = psm.tile([P, 512], F32, tag="mps",
                                              name="projps")
                                for kd in range(8):
                                    nc.tensor.matmul(
                                        ps[:],
                                        wsb[:, kd * DH + dot * P : kd * DH + (dot + 1) * P],
                                        xsb[:, kd * 1024 + tc2 * 512 : kd * 1024 + (tc2 + 1) * 512],
                                        start=(kd == 0),
                                        stop=(kd == 7),
                                    )
                                evac(ps, dot, th * 1024 + tc2 * 512)
                                if pull is not None:
                                    pull()
                with (
                    tc.tile_pool(name="projp", bufs=2) as jp,
                    tc.tile_pool(name="projw", bufs=2) as jw,
                    tc.tile_pool(name="rhtmp", bufs=1) as jr,
                ):
                    wsb = load_w(jw, Wq)

                    def evac_q(ps, dot, col):
                        nc.vector.tensor_scalar(
                            quT[:, dot * S + col : dot * S + col + 512], ps[:],
                            ub_sb[:, dot : dot + 1], None, OP.add)
                        nc.vector.tensor_scalar(
                            qvT[:, dot * S + col : dot * S + col + 512], ps[:],
                            vb_sb[:, dot : dot + 1], None, OP.add)

                    proj_T(jp, wsb, qT, S, evac_q)

                    rhT = jr.tile([P, 4 * T], F16, tag="rhT")
                    wsb = load_w(jw, Wr)

                    def evac_r(ps, dot, col):
                        nc.vector.tensor_copy(
                            rhT[:, dot * T + col : dot * T + col + 512], ps[:])

                    proj_T(jp, wsb, RT, T, evac_r)

                    # rh2T[:, m'] = rhT[:, (m' + 1023) % 2048], m' in [0, 3072)
                    for dot in range(4):
                        nc.vector.tensor_copy(
                            rh2T[:, dot * 3072 : dot * 3072 + 1025],
                            rhT[:, dot * T + 1023 : dot * T + 2048])
                        nc.vector.tensor_copy(
                            rh2T[:, dot * 3072 + 1025 : dot * 3072 + 3072],
                            rhT[:, dot * T : dot * T + 2047])

                    # k and v projections, with hp0's M stage pipelined in
                    units0 = m_units_for(0, split_evac=True)
                    pstate = {"i": 0, "chain": 0}
                    nchains = (T // 1024) * 8 + 2 * 8  # k chains + v chains

                    def pull():
                        pstate["chain"] += 1
                        tgt = min(len(units0),
                                  pstate["chain"] * len(units0) // nchains)
                        while pstate["i"] < tgt:
                            units0[pstate["i"]]()
                            pstate["i"] += 1

                    wsb = load_w(jw, Wk)

                    def evac_k(ps, dot, col):
                        nc.vector.tensor_copy(
                            khT[:, dot * T + col : dot * T + col + 512], ps[:])

                    proj_T(jp, wsb, kT, T, evac_k, pull=pull)

                    # vh (untransposed): per key tile tt, psum [128 keys, 512 dh]
                    wsb = load_w(jw, Wv)
                    for tg in range(2):
                        vsb = load_x(jp, vT, tg)
                        for tl in range(8):
                            ps = psm.tile([P, 512], F32, tag="mps", name="vhps")
                            for kd in range(8):
                                nc.tensor.matmul(
                                    ps[:],
                                    vsb[:, kd * 1024 + tl * P : kd * 1024 + (tl + 1) * P],
                                    wsb[:, kd * DH : (kd + 1) * DH],
                                    start=(kd == 0),
                                    stop=(kd == 7),
                                )
                            tt = tg * 8 + tl
                            base = tt * (HC * 65)
                            dst = bass.AP(vh16.tensor, vh16.offset + base,
                                          [[vh16.tensor.shape[1], P], [65, HC], [1, DK]])
                            nc.vector.tensor_copy(
                                dst, ps[:].rearrange("p (h c) -> p h c", h=HC))
                            ones = bass.AP(vh16.tensor, vh16.offset + base + DK,
                                           [[vh16.tensor.shape[1], P], [65, HC]])
                            nc.vector.memset(ones, 1.0)
                            pull()
                    while pstate["i"] < len(units0):
                        units0[pstate["i"]]()
                        pstate["i"] += 1

                # ---------------- attention quad stages ----------------
                with (
                    tc.tile_pool(name="att_att", bufs=8) as atp,
                    tc.tile_pool(name="att_ebd", bufs=12) as bp,
                    tc.tile_pool(name="nrm", bufs=2) as np_,
                    tc.tile_pool(name="nrmb", bufs=2) as npb,
                ):
                    def quad_stage(hp, nxt_units):
                        mreg = mregs[hp]
                        upos = 0
                        bi = 0
                        for Q in range(2):
                            # dense burst to re-arm the HAM clock gate
                            kal = psac.tile([P, 512], F32, tag="acps",
                                            name="kal")
                            for _ in range(8):
                                nc.tensor.matmul(
                                    kal[:], ident_sb[:], khT[0:P, 0:512],
                                    start=True, stop=True)

                            avps = [psav.tile([P, 512], F32, tag="avps",
                                              name="avps") for _ in range(2)]
                            nkb = 12 if Q == 0 else 16
                            acs = {}
                            ebds = {}

                            def emit_read(qi, Q=Q, ebds=ebds, mreg=mreg):
                                # plain sheared read: bd[il, j] = M[il, il+j],
                                # rows at stride W+1 against the stride-W write
                                Wq_ = w_of(qi)
                                KQ = kq_of(qi)
                                base = SLOT_OFF[Q][qi - 4 * Q]
                                for h in range(2):
                                    ebd = bp.tile([P, 2176], F16, tag="ebd",
                                                  name="ebd")
                                    reg = mreg[(h, Q)]
                                    eng = nc.scalar if h else nc.sync
                                    eng.dma_start(
                                        out=ebd[:, :KQ],
                                        in_=bass.AP(
                                            reg.tensor, reg.offset + base,
                                            [[Wq_ + 1, P], [1, KQ]]))
                                    ebds[(h, qi)] = ebd

                            def emit_ac(kb, Q=Q, acs=acs, hp=hp):
                                for h in range(2):
                                    pr = slice(h * DK, (h + 1) * DK)
                                    qs = max(4 * Q, kb - 8)
                                    n_q = (4 * Q + 4 - qs) * P
                                    acps = psac.tile([P, 512], F32,
                                                     tag="acps", name="acps")
                                    nc.tensor.matmul(
                                        acps[:, :n_q],
                                        khT[pr, hp * T + kb * P :
                                            hp * T + (kb + 1) * P],
                                        quT[pr, hp * S + qs * P :
                                            hp * S + qs * P + n_q],
                                        start=True, stop=False,
                                        skip_group_check=True,
                                    )
                                    acs[(h, kb)] = (acps, qs, n_q)

                            def emit_add(kb, Q=Q, acs=acs, ebds=ebds):
                                # acps[k, q] += bd^T: bd tile as the
                                # stationary operand, identity moving
                                for h in range(2):
                                    acps, qs, n_q = acs[(h, kb)]
                                    for qi in range(qs, 4 * Q + 4):
                                        qloc = (qi - qs) * P
                                        nc.tensor.matmul(
                                            acps[:, qloc : qloc + P],
                                            ebds[(h, qi)][:, kb * P : (kb + 1) * P],
                                            ident_sb[:],
                                            start=False,
                                            stop=(qi == 4 * Q + 3),
                                            skip_group_check=True,
                                        )

                            atts = {}

                            def emit_exp(kb, Q=Q, acs=acs, atts=atts):
                                for h in range(2):
                                    acps, qs, n_q = acs.pop((h, kb))
                                    att = atp.tile([P, 512], F16, tag="att",
                                                   name="att")
                                    nc.scalar.activation(
                                        att[:, :n_q], acps[:, :n_q], AF.Exp,
                                        bias=biasn_sb[:], scale=0.125)
                                    if kb >= 4 * Q + 8:
                                        nc.vector.copy_predicated(
                                            att[:, 0:P], atril2_sb[:],
                                            zeros_sb[:])
                                    atts[(h, kb)] = (att, qs, n_q)

                            def emit_avmm(kb, last, Q=Q, atts=atts,
                                          avps=avps, hp=hp):
                                for h in range(2):
                                    att, qs, n_q = atts.pop((h, kb))
                                    qloc = (qs - 4 * Q) * P
                                    col = kb * (HC * 65) + (hp * 2 + h) * 65
                                    nc.tensor.matmul(
                                        avps[h][0:65, qloc : qloc + n_q],
                                        vh16[:, col : col + 65],
                                        att[:, :n_q],
                                        start=(kb == 0), stop=last,
                                        skip_group_check=True,
                                    )

                            for qi in range(4 * Q, 4 * Q + 4):
                                emit_read(qi)
                            emit_ac(0)
                            emit_add(0)
                            for i in range(nkb):
                                bi += 1
                                emit_exp(i)
                                if i + 1 < nkb:
                                    emit_ac(i + 1)
                                # finish the next hp's M stage by ~beat 20
                                # so its writes land well before the reads
                                target = bi * len(nxt_units) // 20
                                while upos < min(target, len(nxt_units)):
                                    nxt_units[upos]()
                                    upos += 1
                                emit_avmm(i, last=(i == nkb - 1))
                                if i + 1 < nkb:
                                    emit_add(i + 1)
                            for qi in range(4 * Q, 4 * Q + 4):
                                del ebds[(0, qi)], ebds[(1, qi)]

                            # seam drain: PE work with no new deps, emitted
                            # before the norm's recip-gated matmuls
                            seam_target = ((bi + 8) * len(nxt_units) // 20
                                           if Q == 0 else len(nxt_units))
                            while upos < min(seam_target, len(nxt_units)):
                                nxt_units[upos]()
                                upos += 1

                            # ---- normalization for this quad ----
                            dcol = np_.tile([P, 512], F32, tag="dcol",
                                            name="dcol")
                            rcol = np_.tile([P, 512], F32, tag="rcol",
                                            name="rcol")
                            nc.vector.memset(dcol[:], 1.0)
                            for h in range(2):
                                c = 32 * h
                                nc.scalar.activation(
                                    dcol[c : c + 1, :],
                                    avps[h][64:65, :], AF.Copy)
                            nc.vector.reciprocal(rcol[:], dcol[:])
                            for h in range(2):
                                c = 32 * h
                                rbp = psac.tile([P, 512], F32, tag="acps",
                                                name="rbp")
                                nc.tensor.matmul(
                                    rbp[0:DK, :], ones_sb[c : c + 1, :],
                                    rcol[c : c + 1, :], start=True, stop=True,
                                    tile_position=(c, 0))
                                rbc = npb.tile([DK, 512], F32, tag="rbc")
                                nc.vector.tensor_copy(rbc[:], rbp[0:DK, :])
                                nc.vector.tensor_tensor(
                                    concatT[h * DK : (h + 1) * DK,
                                            hp * S + Q * 512 :
                                            hp * S + (Q + 1) * 512],
                                    avps[h][0:DK, :], rbc[:], OP.mult)
                        while upos < len(nxt_units):
                            nxt_units[upos]()
                            upos += 1

                    for hp in range(4):
                        nxt = (m_units_for(hp + 1, split_evac=False)
                               if hp < 3 else [])
                        quad_stage(hp, nxt)

            # ---------------- output projection ----------------
            with (
                tc.tile_pool(name="outp", bufs=2) as op_,
                tc.tile_pool(name="outpsum", bufs=4, space="PSUM") as ops_,
            ):
                for it in range(8):
                    osb = op_.tile([P, 1024], F32, tag="osb")
                    for oc in range(2):
                        ps = ops_.tile([P, 512], F32, tag="out")
                        for dt in range(4):
                            nc.tensor.matmul(
                                ps[:],
                                concatT[:, dt * S + it * P : dt * S + (it + 1) * P],
                                WoS[:, dt * D + oc * 512 : dt * D + (oc + 1) * 512],
                                start=(dt == 0), stop=(dt == 3),
                            )
                        nc.vector.tensor_copy(osb[:, oc * 512 : (oc + 1) * 512], ps[:])
                    nc.gpsimd.dma_start(
                        out=outp[it * P : (it + 1) * P, :], in_=osb[:])

    if split_waits:
        _split_multi_waits(nc)
    return nc


def prep_core_inputs(core, q, k, v, u, v_bias, Wq, Wk, Wv, Wr, Wo, R):
    b, hh = core // 2, core % 2
    sl = slice(hh * DH, (hh + 1) * DH)
    BF = ml_dtypes.bfloat16
    return {
        "qT": q[b].T.astype(BF),
        "kT": k[b].T.astype(BF),
        "vT": v[b].T.astype(BF),
        "RT": R.T.astype(BF),
        "Wq": Wq[sl, :].T.astype(BF),
        "Wk": Wk[sl, :].T.astype(BF),
        "Wv": Wv[sl, :].T.astype(BF),
        "Wr": Wr[sl, :].T.astype(BF),
        "Wo16": Wo[:, sl].T.astype(np.float16),
        "ub": np.ascontiguousarray(
            u[0, hh * HC : (hh + 1) * HC, 0, :].reshape(4, P).T),
        "vb": np.ascontiguousarray(
            v_bias[0, hh * HC : (hh + 1) * HC, 0, :].reshape(4, P).T),
        "atril2": np.tril(np.ones((P, P), np.uint8), -1),
        "ident": np.eye(P, dtype=np.float16),
    }


def combine_outputs(results):
    # results: list of 8 dicts with "out" [S, D]; partial sums per batch pair
    out = np.empty((4, S, D), np.float32)
    for b in range(4):
        out[b] = results[2 * b]["out"] + results[2 * b + 1]["out"]
    return out


_CACHED_NC = None
last_result = None  # BassKernelResults of the most recent run (for test harness)


def kernel(q, k, v, mask, u, v_bias, Wq, Wk, Wv, Wr, Wo, R):
    global _CACHED_NC, last_result
    from concourse.bass_utils import run_bass_kernel_spmd

    q, k, v = np.asarray(q), np.asarray(k), np.asarray(v)
    u, v_bias = np.asarray(u), np.asarray(v_bias)
    Wq, Wk, Wv, Wr, Wo, R = map(np.asarray, (Wq, Wk, Wv, Wr, Wo, R))

    # The kernel exploits the known TXL mask structure (j <= i + MEM).
    # Verify the passed mask matches; structural masking is baked in.
    m = np.asarray(mask)
    exp_mask = (np.arange(T)[None, :] <= np.arange(S)[:, None] + 1024)
    assert m.shape == (4, S, T) and bool((m == exp_mask[None]).all()), \
        "kernel compiled for the TXL causal mask (j <= i + MEM)"

    if _CACHED_NC is None:
        _CACHED_NC = build_nc()

    in_maps = [prep_core_inputs(c, q, k, v, u, v_bias, Wq, Wk, Wv, Wr, Wo, R)
               for c in range(8)]
    trace = bool(os.environ.get("TXL_TRACE"))
    last_result = run_bass_kernel_spmd(
        _CACHED_NC, in_maps, list(range(8)), trace=trace,
        trace_cores=[0] if trace else None)
    return combine_outputs(last_result.results)


# revision 20
# speedup vs baseline: 1.0503x; 1.0503x over previous
"""Transformer-XL relative multi-head attention, 8-way sharded on Trainium2.

Self-contained harness entry: kernel(**inputs) -> np.ndarray [4, 1024, 1024].

Sharding: core c handles batch b = c//2 and head-half hh = c%2 (8 of 16
heads). Each core computes a partial output (its heads' contribution
through Wo); the host unshard sums the two partials per batch (row-parallel
tensor parallelism for the output projection).

Kernel structure ("transposed-scores" form):
  - projections q/k/v/R (bf16 inputs, batched DMA loads)
  - position matrix M = (qh+v)^T . rh2 per query tile, exp'd on the scalar
    engine, written to a DRAM scratch region with row stride W; the
    circulant shift is realized by reading back with row stride W+1
    through a single DMA-transpose per (quad, key-block), which lands
    exp(bd)^T [key, query] tiles directly in SBUF.
  - content scores computed transposed (ac^T = kh^T-block . qu) so the
    AV matmul needs no attention transpose at all; exp on scalar engine.
  - att = exp(ac)*exp(bd) (one fp16 vector multiply); AV accumulates over
    key blocks with an appended ones-column in vh to produce the softmax
    denominator; normalization applied after AV via a partition-broadcast
    reciprocal.
  - heads h0/h1 of each pair are issued back-to-back so their K=64
    matmuls pack into the two row-halves of the PE array.
"""

import os
import sys

sys.path.insert(0, "/opt/trn_rl_repo")

import numpy as np
import ml_dtypes

import concourse.bass as bass
import concourse.mybir as mybir
from concourse.tile import TileContext, ScopedClock

F32 = mybir.dt.float32
F16 = mybir.dt.float16
BF16 = mybir.dt.bfloat16
AF = mybir.ActivationFunctionType
OP = mybir.AluOpType

S, T, D, DK, P = 1024, 2048, 1024, 64, 128
DH = 512          # head-slice width per core (8 heads)
HC = 8
EXP_BIAS = -7.0   # bias inside the single exp (cancels in softmax)
def _w_of(qi):
    return (qi + 9) * 128 + 127


SLOT = [128 * (_w_of(qi) + 1) for qi in range(8)]   # per-qi scratch slot elems
SLOT_OFF = [[sum(SLOT[4 * Q : 4 * Q + j]) for j in range(4)] for Q in range(2)]
QREG = [sum(SLOT[4 * Q : 4 * Q + 4]) for Q in range(2)]  # per-(h,Q) region


def kq_of(qi):  # valid key count for query tile qi (keys j <= i + 1024)
    return (qi + 9) * P


def w_of(qi):  # position-matrix width for query tile qi
    return kq_of(qi) + 127


def cdiv(a, b):
    return (a + b - 1) // b


def _patched_drain_and_barrier(self, tick_clock, wait_clock):
    # The walrus build in this container caps sync-waits per instruction;
    # Tile's stock tail drain carries one wait per live proc. Emit one SP nop
    # per wait instead, then the drain.
    dummy = mybir.InstNoOp(name="drain-wait-probe", ins=[], outs=[])
    dummy.engine = mybir.EngineType.SP
    wait_clock.add_sem_waits(dummy, ScopedClock({None: tick_clock.global_clock}))
    waits = []
    if dummy.sync_info is not None and dummy.sync_info.on_wait:
        waits = [(w.ant_name, w.wait_value) for w in dummy.sync_info.on_wait]
    assert self.sems is not None
    name2sem = {h.name: h for h in self.sems.allocated().values()}
    for name, val in waits:
        self.nc.sync.nop().wait_op(name2sem[name], val, "sem-ge")
    self.nc.sync.drain()
    self.nc.all_engine_barrier()
    popped = self.nc._tile_sem_poison_stack.pop()
    assert popped is self._sem_poison
    self.nc.clear_and_free_semaphores(list(self.sems.allocated().values()))
    self.nc.all_engine_barrier()


TileContext._drain_and_barrier = _patched_drain_and_barrier


def _split_multi_waits(nc, max_waits=1):
    """Walrus in this container rejects instructions carrying more than a
    couple of sync waits. Hoist extras onto same-engine NoOps just before
    the instruction (sequential on the engine, so semantics unchanged)."""
    for f in nc.m.functions:
        for bb in f.blocks:
            out = []
            changed = False
            for inst in bb.instructions:
                si = inst.sync_info
                if si is not None and si.on_wait and len(si.on_wait) > max_waits:
                    waits = list(si.on_wait)
                    for j, w in enumerate(waits[:-max_waits]):
                        nop = mybir.InstNoOp(
                            name=f"{inst.name}-wsplit{j}", ins=[], outs=[])
                        nop.engine = inst.engine
                        nop.sync_info = mybir.SyncInfo(on_wait=[w], on_update=[])
                        out.append(nop)
                    inst.sync_info = mybir.SyncInfo(
                        on_wait=waits[-max_waits:],
                        on_update=list(si.on_update))
                    changed = True
                out.append(inst)
            if changed:
                bb.instructions = out


def build_nc(split_waits=True):
    nc = bass.Bass(target_bir_lowering=True)

    qT = nc.declare_dram_parameter("qT", [D, S], BF16, isOutput=False)
    kT = nc.declare_dram_parameter("kT", [D, T], BF16, isOutput=False)
    vT = nc.declare_dram_parameter("vT", [D, T], BF16, isOutput=False)
    RT = nc.declare_dram_parameter("RT", [D, T], BF16, isOutput=False)
    Wq = nc.declare_dram_parameter("Wq", [D, DH], BF16, isOutput=False)
    Wk = nc.declare_dram_parameter("Wk", [D, DH], BF16, isOutput=False)
    Wv = nc.declare_dram_parameter("Wv", [D, DH], BF16, isOutput=False)
    Wr = nc.declare_dram_parameter("Wr", [D, DH], BF16, isOutput=False)
    Wo16 = nc.declare_dram_parameter("Wo16", [DH, D], F16, isOutput=False)
    ub = nc.declare_dram_parameter("ub", [P, 4], F32, isOutput=False)
    vb = nc.declare_dram_parameter("vb", [P, 4], F32, isOutput=False)
    atril2 = nc.declare_dram_parameter(
        "atril2", [P, P], mybir.dt.uint8, isOutput=False)
    ident = nc.declare_dram_parameter("ident", [P, P], F16, isOutput=False)
    outp = nc.declare_dram_parameter("out", [S, D], F32, isOutput=True)

    with TileContext(nc) as tc:
        with (
            tc.tile_pool(name="persist", bufs=1) as pp,
            tc.tile_pool(name="consts", bufs=1) as cp,
        ):
            # persistent fp16 tensors (partition = dk of the 2 heads in a pair)
            quT = pp.tile([P, 4 * S], F16)       # (qh+u).T   blocks hp
            qvT = pp.tile([P, 4 * S], F16)       # (qh+v).T
            khT = pp.tile([P, 4 * T], F16)
            rh2T = pp.tile([P, 4 * 3072], F16)
            vh16 = pp.tile([P, 16 * (HC * 65)], F16)  # per key tile: 8 h x (64+1)
            concatT = pp.tile([P, 4 * S], F16)
            WoS = pp.tile([P, 4 * D], F16)

            ub_sb = cp.tile([P, 4], F32)
            vb_sb = cp.tile([P, 4], F32)
            atril2_sb = cp.tile([P, P], mybir.dt.uint8)
            zeros_sb = cp.tile([P, P], F16)
            biasn_sb = cp.tile([P, 1], F32)
            ones_sb = cp.tile([P, DK], F32)
            ident_sb = cp.tile([P, P], F16)
            nc.vector.memset(biasn_sb[:], EXP_BIAS)
            nc.vector.memset(zeros_sb[:], 0.0)
            nc.vector.memset(ones_sb[:], 1.0)

            nc.scalar.dma_start(out=ub_sb[:], in_=ub[:])
            nc.scalar.dma_start(out=vb_sb[:], in_=vb[:])
            nc.scalar.dma_start(out=atril2_sb[:], in_=atril2[:])
            nc.scalar.dma_start(out=ident_sb[:], in_=ident[:])
            # WoS layout [128, dt*1024 + o] <- Wo16[(dt p), o], one DMA
            nc.scalar.dma_start(
                out=WoS[:],
                in_=bass.AP(Wo16, 0, [[D, P], [P * D, 4], [1, D]]),
            )

            # ---------------- projections + attention ----------------
            # attention-side pools are opened first so hp0's M stage can be
            # software-pipelined into the k/v projection chains; psm doubles
            # as the projection chain psum pool (PSUM budget: 2+4+2 = 8)
            with (
                tc.tile_pool(name="att_m", bufs=3) as mp,
                tc.tile_pool(name="dram", bufs=10, space="DRAM") as dp,
                tc.tile_pool(name="ps_m", bufs=2, space="PSUM") as psm,
                tc.tile_pool(name="ps_ac", bufs=4, space="PSUM") as psac,
                tc.tile_pool(name="ps_av", bufs=2, space="PSUM") as psav,
            ):
                mregs = {}

                def m_units_for(hp, split_evac):
                    """hp's M stage as a list of small closures, so it can be
                    software-pipelined into other work's emission.
                    One DRAM region per (h, Q) holding that quad's 4 slots."""
                    mregs[hp] = {(h, Q): dp.tile([QREG[Q]], F16, tag="mreg",
                                                 name="mreg")
                                 for h in range(2) for Q in range(2)}
                    units = []
                    msbs_state = {}

                    for qi in range(8):
                        Qd = qi // 4
                        Wq_ = w_of(qi)
                        nwc = cdiv(Wq_, 512)

                        def mk_wc(qi=qi, Qd=Qd, Wq_=Wq_, wc_=None):
                            def f(wc=wc_):
                                if qi not in msbs_state:
                                    msbs_state[qi] = [
                                        mp.tile([P, 2176], F16, tag="msb",
                                                name="msb") for _ in range(2)]
                                nw = min(512, Wq_ - wc * 512)
                                for h in range(2):
                                    pr = slice(h * DK, (h + 1) * DK)
                                    mps = psm.tile([P, 512], F32, tag="mps",
                                                   name="mps")
                                    nc.tensor.matmul(
                                        mps[:, :nw],
                                        qvT[pr, hp * S + qi * P :
                                            hp * S + (qi + 1) * P],
                                        rh2T[pr, hp * 3072 + qi * P + wc * 512 :
                                             hp * 3072 + qi * P + wc * 512 + nw],
                                        start=True, stop=True,
                                    )
                                    dst = msbs_state[qi][h][
                                        :, wc * 512 : wc * 512 + nw]
                                    if split_evac and (qi + h) % 2 == 0:
                                        nc.scalar.activation(
                                            dst, mps[:, :nw], AF.Copy)
                                    else:
                                        nc.vector.tensor_copy(dst, mps[:, :nw])
                            return f

                        for wc in range(nwc):
                            units.append(mk_wc(wc_=wc))

                        def mk_wr(qi=qi, Qd=Qd, Wq_=Wq_, hp=hp):
                            def f():
                                msbs = msbs_state.pop(qi)
                                base = SLOT_OFF[Qd][qi % 4]
                                for h in range(2):
                                    reg = mregs[hp][(h, Qd)]
                                    nc.gpsimd.dma_start(
                                        out=bass.AP(
                                            reg.tensor, reg.offset + base,
                                            [[Wq_, P], [1, Wq_]]),
                                        in_=msbs[h][:, :Wq_],
                                    )
                            return f

                        units.append(mk_wr())
                    return units

                def load_w(pool, wparam):
                    wsb = pool.tile([P, 8 * DH], BF16, tag="wsb")
                    nc.sync.dma_start(
                        out=wsb[:],
                        in_=bass.AP(wparam, 0, [[DH, P], [P * DH, 8], [1, DH]]),
                    )
                    return wsb

                def load_x(pool, xparam, th):
                    # 8 DMAs alternating the two HWDGE rings so the first
                    # chain can start after one chunk lands
                    xsb = pool.tile([P, 8 * 1024], BF16, tag="xstage")
                    L = xparam.shape[1]
                    for kd in range(8):
                        eng = nc.scalar if kd % 2 else nc.sync
                        eng.dma_start(
                            out=xsb[:, kd * 1024 : (kd + 1) * 1024],
                            in_=bass.AP(xparam, kd * P * L + th * 1024,
                                        [[L, P], [1, 1024]]),
                        )
                    return xsb

                def proj_T(pool, wsb, xparam, ncols, evac, pull=None):
                    # chain-major: 8 back-to-back matmuls per psum chain
                    nth = ncols // 1024
                    for th in range(nth):
                        xsb = load_x(pool, xparam, th)
                        for dot in range(4):
                            for tc2 in range(2):
                                ps = psm.tile([P, 512], F32, tag="mps",
                                              name="projps")
                                for kd in range(8):
                                    nc.tensor.matmul(
                                        ps[:],
                                        wsb[:, kd * DH + dot * P : kd * DH + (dot + 1) * P],
                                        xsb[:, kd * 1024 + tc2 * 512 : kd * 1024 + (tc2 + 1) * 512],
                                        start=(kd == 0),
                                        stop=(kd == 7),
                                    )
                                evac(ps, dot, th * 1024 + tc2 * 512)
                                if pull is not None:
                                    pull()
                with (
                    tc.tile_pool(name="projp", bufs=2) as jp,
                    tc.tile_pool(name="projw", bufs=2) as jw,
                    tc.tile_pool(name="rhtmp", bufs=1) as jr,
                ):
                    wsb = load_w(jw, Wq)

                    def evac_q(ps, dot, col):
                        nc.vector.tensor_scalar(
                            quT[:, dot * S + col : dot * S + col + 512], ps[:],
                            ub_sb[:, dot : dot + 1], None, OP.add)
                        nc.vector.tensor_scalar(
                            qvT[:, dot * S + col : dot * S + col + 512], ps[:],
                            vb_sb[:, dot : dot + 1], None, OP.add)

                    proj_T(jp, wsb, qT, S, evac_q)

                    rhT = jr.tile([P, 4 * T], F16, tag="rhT")
                    wsb = load_w(jw, Wr)

                    def evac_r(ps, dot, col):
                        nc.vector.tensor_copy(
                            rhT[:, dot * T + col : dot * T + col + 512], ps[:])

                    proj_T(jp, wsb, RT, T, evac_r)

                    # rh2T[:, m'] = rhT[:, (m' + 1023) % 2048], m' in [0, 3072)
                    for dot in range(4):
                        nc.vector.tensor_copy(
                            rh2T[:, dot * 3072 : dot * 3072 + 1025],
                            rhT[:, dot * T + 1023 : dot * T + 2048])
                        nc.vector.tensor_copy(
                            rh2T[:, dot * 3072 + 1025 : dot * 3072 + 3072],
                            rhT[:, dot * T : dot * T + 2047])

                    # k and v projections, with hp0's M stage pipelined in
                    units0 = m_units_for(0, split_evac=True)
                    pstate = {"i": 0, "chain": 0}
                    nchains = (T // 1024) * 8 + 2 * 8  # k chains + v chains

                    def pull():
                        pstate["chain"] += 1
                        tgt = min(len(units0),
                                  pstate["chain"] * len(units0) // nchains)
                        while pstate["i"] < tgt:
                            units0[pstate["i"]]()
                            pstate["i"] += 1

                    wsb = load_w(jw, Wk)

                    def evac_k(ps, dot, col):
                        nc.vector.tensor_copy(
                            khT[:, dot * T + col : dot * T + col + 512], ps[:])

                    proj_T(jp, wsb, kT, T, evac_k, pull=pull)

                    # vh (untransposed): per key tile tt, psum [128 keys, 512 dh]
                    wsb = load_w(jw, Wv)
                    for tg in range(2):
                        vsb = load_x(jp, vT, tg)
                        for tl in range(8):
                            ps = psm.tile([P, 512], F32, tag="mps", name="vhps")
                            for kd in range(8):
                                nc.tensor.matmul(
                                    ps[:],
                                    vsb[:, kd * 1024 + tl * P : kd * 1024 + (tl + 1) * P],
                                    wsb[:, kd * DH : (kd + 1) * DH],
                                    start=(kd == 0),
                                    stop=(kd == 7),
                                )
                            tt = tg * 8 + tl
                            base = tt * (HC * 65)
                            dst = bass.AP(vh16.tensor, vh16.offset + base,
                                          [[vh16.tensor.shape[1], P], [65, HC], [1, DK]])
                            nc.vector.tensor_copy(
                                dst, ps[:].rearrange("p (h c) -> p h c", h=HC))
                            ones = bass.AP(vh16.tensor, vh16.offset + base + DK,
                                           [[vh16.tensor.shape[1], P], [65, HC]])
                            nc.vector.memset(ones, 1.0)
                            pull()
                    while pstate["i"] < len(units0):
                        units0[pstate["i"]]()
                        pstate["i"] += 1

                # ---------------- attention quad stages ----------------
                with (
                    tc.tile_pool(name="att_att", bufs=6) as atp,
                    tc.tile_pool(name="att_ebd", bufs=18) as bp,
                    tc.tile_pool(name="nrm", bufs=2) as np_,
                    tc.tile_pool(name="nrmb", bufs=2) as npb,
                ):
                    def quad_stage(hp, nxt_units):
                        mreg = mregs[hp]
                        upos = 0
                        bi = 0
                        for Q in range(2):
                            # dense burst to re-arm the HAM clock gate
                            kal = psac.tile([P, 512], F32, tag="acps",
                                            name="kal")
                            for _ in range(8):
                                nc.tensor.matmul(
                                    kal[:], ident_sb[:], khT[0:P, 0:512],
                                    start=True, stop=True)

                            avps = [psav.tile([P, 512], F32, tag="avps",
                                              name="avps") for _ in range(2)]
                            nkb = 12 if Q == 0 else 16
                            acs = {}
                            ebds = {}

                            def emit_read(qi, Q=Q, ebds=ebds, mreg=mreg):
                                # plain sheared read: bd[il, j] = M[il, il+j],
                                # rows at stride W+1 against the stride-W write
                                Wq_ = w_of(qi)
                                KQ = kq_of(qi)
                                base = SLOT_OFF[Q][qi - 4 * Q]
                                for h in range(2):
                                    ebd = bp.tile([P, 2048], F16, tag="ebd",
                                                  name="ebd")
                                    reg = mreg[(h, Q)]
                                    eng = nc.scalar if h else nc.sync
                                    eng.dma_start(
                                        out=ebd[:, :KQ],
                                        in_=bass.AP(
                                            reg.tensor, reg.offset + base,
                                            [[Wq_ + 1, P], [1, KQ]]))
                                    ebds[(h, qi)] = ebd

                            def emit_ac(kb, Q=Q, acs=acs, hp=hp):
                                for h in range(2):
                                    pr = slice(h * DK, (h + 1) * DK)
                                    qs = max(4 * Q, kb - 8)
                                    n_q = (4 * Q + 4 - qs) * P
                                    acps = psac.tile([P, 512], F32,
                                                     tag="acps", name="acps")
                                    nc.tensor.matmul(
                                        acps[:, :n_q],
                                        khT[pr, hp * T + kb * P :
                                            hp * T + (kb + 1) * P],
                                        quT[pr, hp * S + qs * P :
                                            hp * S + qs * P + n_q],
                                        start=True, stop=False,
                                        skip_group_check=True,
                                    )
                                    acs[(h, kb)] = (acps, qs, n_q)

                            def emit_add(kb, Q=Q, acs=acs, ebds=ebds):
                                # acps[k, q] += bd^T: bd tile as the
                                # stationary operand, identity moving
                                for h in range(2):
                                    acps, qs, n_q = acs[(h, kb)]
                                    for qi in range(qs, 4 * Q + 4):
                                        qloc = (qi - qs) * P
                                        nc.tensor.matmul(
                                            acps[:, qloc : qloc + P],
                                            ebds[(h, qi)][:, kb * P : (kb + 1) * P],
                                            ident_sb[:],
                                            start=False,
                                            stop=(qi == 4 * Q + 3),
                                            skip_group_check=True,
                                        )

                            atts = {}

                            def emit_exp(kb, Q=Q, acs=acs, atts=atts):
                                for h in range(2):
                                    acps, qs, n_q = acs.pop((h, kb))
                                    att = atp.tile([P, 512], F16, tag="att",
                                                   name="att")
                                    nc.scalar.activation(
                                        att[:, :n_q], acps[:, :n_q], AF.Exp,
                                        bias=biasn_sb[:], scale=0.125)
                                    if kb >= 4 * Q + 8:
                                        nc.vector.copy_predicated(
                                            att[:, 0:P], atril2_sb[:],
                                            zeros_sb[:])
                                    atts[(h, kb)] = (att, qs, n_q)

                            def emit_avmm(kb, last, Q=Q, atts=atts,
                                          avps=avps, hp=hp):
                                for h in range(2):
                                    att, qs, n_q = atts.pop((h, kb))
                                    qloc = (qs - 4 * Q) * P
                                    col = kb * (HC * 65) + (hp * 2 + h) * 65
                                    nc.tensor.matmul(
                                        avps[h][0:65, qloc : qloc + n_q],
                                        vh16[:, col : col + 65],
                                        att[:, :n_q],
                                        start=(kb == 0), stop=last,
                                        skip_group_check=True,
                                    )

                            for qi in range(4 * Q, 4 * Q + 4):
                                emit_read(qi)
                            emit_ac(0)
                            emit_add(0)
                            for i in range(nkb):
                                bi += 1
                                emit_exp(i)
                                if i + 1 < nkb:
                                    emit_ac(i + 1)
                                # finish the next hp's M stage by ~beat 20
                                # so its writes land well before the reads
                                target = bi * len(nxt_units) // 24
                                while upos < min(target, len(nxt_units)):
                                    nxt_units[upos]()
                                    upos += 1
                                emit_avmm(i, last=(i == nkb - 1))
                                if i + 1 < nkb:
                                    emit_add(i + 1)
                            for qi in range(4 * Q, 4 * Q + 4):
                                del ebds[(0, qi)], ebds[(1, qi)]

                            # seam drain: PE work with no new deps, emitted
                            # before the norm's recip-gated matmuls
                            seam_target = ((bi + 8) * len(nxt_units) // 24
                                           if Q == 0 else len(nxt_units))
                            while upos < min(seam_target, len(nxt_units)):
                                nxt_units[upos]()
                                upos += 1

                            # ---- normalization for this quad ----
                            dcol = np_.tile([P, 512], F32, tag="dcol",
                                            name="dcol")
                            rcol = np_.tile([P, 512], F32, tag="rcol",
                                            name="rcol")
                            nc.vector.memset(dcol[:], 1.0)
                            for h in range(2):
                                c = 32 * h
                                nc.scalar.activation(
                                    dcol[c : c + 1, :],
                                    avps[h][64:65, :], AF.Copy)
                            nc.vector.reciprocal(rcol[:], dcol[:])
                            for h in range(2):
                                c = 32 * h
                                rbp = psac.tile([P, 512], F32, tag="acps",
                                                name="rbp")
                                nc.tensor.matmul(
                                    rbp[0:DK, :], ones_sb[c : c + 1, :],
                                    rcol[c : c + 1, :], start=True, stop=True,
                                    tile_position=(c, 0))
                                rbc = npb.tile([DK, 512], F32, tag="rbc")
                                nc.vector.tensor_copy(rbc[:], rbp[0:DK, :])
                                nc.vector.tensor_tensor(
                                    concatT[h * DK : (h + 1) * DK,
                                            hp * S + Q * 512 :
                                            hp * S + (Q + 1) * 512],
                                    avps[h][0:DK, :], rbc[:], OP.mult)
                        while upos < len(nxt_units):
                            nxt_units[upos]()
                            upos += 1

                    for hp in range(4):
                        nxt = (m_units_for(hp + 1, split_evac=False)
                               if hp < 3 else [])
                        quad_stage(hp, nxt)

            # ---------------- output projection ----------------
            with (
                tc.tile_pool(name="outp", bufs=2) as op_,
                tc.tile_pool(name="outpsum", bufs=4, space="PSUM") as ops_,
            ):
                for it in range(8):
                    osb = op_.tile([P, 1024], F32, tag="osb")
                    for oc in range(2):
                        ps = ops_.tile([P, 512], F32, tag="out")
                        for dt in range(4):
                            nc.tensor.matmul(
                                ps[:],
                                concatT[:, dt * S + it * P : dt * S + (it + 1) * P],
                                WoS[:, dt * D + oc * 512 : dt * D + (oc + 1) * 512],
                                start=(dt == 0), stop=(dt == 3),
                            )
                        nc.vector.tensor_copy(osb[:, oc * 512 : (oc + 1) * 512], ps[:])
                    nc.gpsimd.dma_start(
                        out=outp[it * P : (it + 1) * P, :], in_=osb[:])

    if split_waits:
        _split_multi_waits(nc)
    return nc


def prep_core_inputs(core, q, k, v, u, v_bias, Wq, Wk, Wv, Wr, Wo, R):
    b, hh = core // 2, core % 2
    sl = slice(hh * DH, (hh + 1) * DH)
    BF = ml_dtypes.bfloat16
    return {
        "qT": q[b].T.astype(BF),
        "kT": k[b].T.astype(BF),
        "vT": v[b].T.astype(BF),
        "RT": R.T.astype(BF),
        "Wq": Wq[sl, :].T.astype(BF),
        "Wk": Wk[sl, :].T.astype(BF),
        "Wv": Wv[sl, :].T.astype(BF),
        "Wr": Wr[sl, :].T.astype(BF),
        "Wo16": Wo[:, sl].T.astype(np.float16),
        "ub": np.ascontiguousarray(
            u[0, hh * HC : (hh + 1) * HC, 0, :].reshape(4, P).T),
        "vb": np.ascontiguousarray(
            v_bias[0, hh * HC : (hh + 1) * HC, 0, :].reshape(4, P).T),
        "atril2": np.tril(np.ones((P, P), np.uint8), -1),
        "ident": np.eye(P, dtype=np.float16),
    }


def combine_outputs(results):
    # results: list of 8 dicts with "out" [S, D]; partial sums per batch pair
    out = np.empty((4, S, D), np.float32)
    for b in range(4):
        out[b] = results[2 * b]["out"] + results[2 * b + 1]["out"]
    return out


_CACHED_NC = None
last_result = None  # BassKernelResults of the most recent run (for test harness)


def kernel(q, k, v, mask, u, v_bias, Wq, Wk, Wv, Wr, Wo, R):
    global _CACHED_NC, last_result
    from concourse.bass_utils import run_bass_kernel_spmd

    q, k, v = np.asarray(q), np.asarray(k), np.asarray(v)
    u, v_bias = np.asarray(u), np.asarray(v_bias)
    Wq, Wk, Wv, Wr, Wo, R = map(np.asarray, (Wq, Wk, Wv, Wr, Wo, R))

    # The kernel exploits the known TXL mask structure (j <= i + MEM).
    # Verify the passed mask matches; structural masking is baked in.
    m = np.asarray(mask)
    exp_mask = (np.arange(T)[None, :] <= np.arange(S)[:, None] + 1024)
    assert m.shape == (4, S, T) and bool((m == exp_mask[None]).all()), \
        "kernel compiled for the TXL causal mask (j <= i + MEM)"

    if _CACHED_NC is None:
        _CACHED_NC = build_nc()

    in_maps = [prep_core_inputs(c, q, k, v, u, v_bias, Wq, Wk, Wv, Wr, Wo, R)
               for c in range(8)]
    trace = bool(os.environ.get("TXL_TRACE"))
    last_result = run_bass_kernel_spmd(
        _CACHED_NC, in_maps, list(range(8)), trace=trace,
        trace_cores=[0] if trace else None)
    return combine_outputs(last_result.results)


# revision 21
# speedup vs baseline: 1.0733x; 1.0218x over previous
"""Transformer-XL relative multi-head attention, 8-way sharded on Trainium2.

Self-contained harness entry: kernel(**inputs) -> np.ndarray [4, 1024, 1024].

Sharding: core c handles batch b = c//2 and head-half hh = c%2 (8 of 16
heads). Each core computes a partial output (its heads' contribution
through Wo); the host unshard sums the two partials per batch (row-parallel
tensor parallelism for the output projection).

Kernel structure ("transposed-scores" form):
  - projections q/k/v/R (bf16 inputs, batched DMA loads)
  - position matrix M = (qh+v)^T . rh2 per query tile, exp'd on the scalar
    engine, written to a DRAM scratch region with row stride W; the
    circulant shift is realized by reading back with row stride W+1
    through a single DMA-transpose per (quad, key-block), which lands
    exp(bd)^T [key, query] tiles directly in SBUF.
  - content scores computed transposed (ac^T = kh^T-block . qu) so the
    AV matmul needs no attention transpose at all; exp on scalar engine.
  - att = exp(ac)*exp(bd) (one fp16 vector multiply); AV accumulates over
    key blocks with an appended ones-column in vh to produce the softmax
    denominator; normalization applied after AV via a partition-broadcast
    reciprocal.
  - heads h0/h1 of each pair are issued back-to-back so their K=64
    matmuls pack into the two row-halves of the PE array.
"""

import os
import sys

sys.path.insert(0, "/opt/trn_rl_repo")

import numpy as np
import ml_dtypes

import concourse.bass as bass
import concourse.mybir as mybir
from concourse.tile import TileContext, ScopedClock

F32 = mybir.dt.float32
F16 = mybir.dt.float16
BF16 = mybir.dt.bfloat16
AF = mybir.ActivationFunctionType
OP = mybir.AluOpType

S, T, D, DK, P = 1024, 2048, 1024, 64, 128
DH = 512          # head-slice width per core (8 heads)
HC = 8
EXP_BIAS = -7.0   # bias inside the single exp (cancels in softmax)
def _w_of(qi):
    return (qi + 9) * 128 + 127


SLOT = [128 * (_w_of(qi) + 1) for qi in range(8)]   # per-qi scratch slot elems
SLOT_OFF = [[sum(SLOT[4 * Q : 4 * Q + j]) for j in range(4)] for Q in range(2)]
QREG = [sum(SLOT[4 * Q : 4 * Q + 4]) for Q in range(2)]  # per-(h,Q) region


def kq_of(qi):  # valid key count for query tile qi (keys j <= i + 1024)
    return (qi + 9) * P


def w_of(qi):  # position-matrix width for query tile qi
    return kq_of(qi) + 127


def cdiv(a, b):
    return (a + b - 1) // b


def _patched_drain_and_barrier(self, tick_clock, wait_clock):
    # The walrus build in this container caps sync-waits per instruction;
    # Tile's stock tail drain carries one wait per live proc. Emit one SP nop
    # per wait instead, then the drain.
    dummy = mybir.InstNoOp(name="drain-wait-probe", ins=[], outs=[])
    dummy.engine = mybir.EngineType.SP
    wait_clock.add_sem_waits(dummy, ScopedClock({None: tick_clock.global_clock}))
    waits = []
    if dummy.sync_info is not None and dummy.sync_info.on_wait:
        waits = [(w.ant_name, w.wait_value) for w in dummy.sync_info.on_wait]
    assert self.sems is not None
    name2sem = {h.name: h for h in self.sems.allocated().values()}
    for name, val in waits:
        self.nc.sync.nop().wait_op(name2sem[name], val, "sem-ge")
    self.nc.sync.drain()
    self.nc.all_engine_barrier()
    popped = self.nc._tile_sem_poison_stack.pop()
    assert popped is self._sem_poison
    self.nc.clear_and_free_semaphores(list(self.sems.allocated().values()))
    self.nc.all_engine_barrier()


TileContext._drain_and_barrier = _patched_drain_and_barrier


def _split_multi_waits(nc, max_waits=1):
    """Walrus in this container rejects instructions carrying more than a
    couple of sync waits. Hoist extras onto same-engine NoOps just before
    the instruction (sequential on the engine, so semantics unchanged)."""
    for f in nc.m.functions:
        for bb in f.blocks:
            out = []
            changed = False
            for inst in bb.instructions:
                si = inst.sync_info
                if si is not None and si.on_wait and len(si.on_wait) > max_waits:
                    waits = list(si.on_wait)
                    for j, w in enumerate(waits[:-max_waits]):
                        nop = mybir.InstNoOp(
                            name=f"{inst.name}-wsplit{j}", ins=[], outs=[])
                        nop.engine = inst.engine
                        nop.sync_info = mybir.SyncInfo(on_wait=[w], on_update=[])
                        out.append(nop)
                    inst.sync_info = mybir.SyncInfo(
                        on_wait=waits[-max_waits:],
                        on_update=list(si.on_update))
                    changed = True
                out.append(inst)
            if changed:
                bb.instructions = out


def build_nc(split_waits=True):
    nc = bass.Bass(target_bir_lowering=True)

    qT = nc.declare_dram_parameter("qT", [D, S], BF16, isOutput=False)
    kT = nc.declare_dram_parameter("kT", [D, T], BF16, isOutput=False)
    vT = nc.declare_dram_parameter("vT", [D, T], BF16, isOutput=False)
    RT = nc.declare_dram_parameter("RT", [D, T], BF16, isOutput=False)
    Wq = nc.declare_dram_parameter("Wq", [D, DH], BF16, isOutput=False)
    Wk = nc.declare_dram_parameter("Wk", [D, DH], BF16, isOutput=False)
    Wv = nc.declare_dram_parameter("Wv", [D, DH], BF16, isOutput=False)
    Wr = nc.declare_dram_parameter("Wr", [D, DH], BF16, isOutput=False)
    Wo16 = nc.declare_dram_parameter("Wo16", [DH, D], F16, isOutput=False)
    ub = nc.declare_dram_parameter("ub", [P, 4], F32, isOutput=False)
    vb = nc.declare_dram_parameter("vb", [P, 4], F32, isOutput=False)
    atril2 = nc.declare_dram_parameter(
        "atril2", [P, P], mybir.dt.uint8, isOutput=False)
    ident = nc.declare_dram_parameter("ident", [P, P], F16, isOutput=False)
    outp = nc.declare_dram_parameter("out", [S, D], F32, isOutput=True)

    with TileContext(nc) as tc:
        with (
            tc.tile_pool(name="persist", bufs=1) as pp,
            tc.tile_pool(name="consts", bufs=1) as cp,
        ):
            # persistent fp16 tensors (partition = dk of the 2 heads in a pair)
            quT = pp.tile([P, 4 * S], F16)       # (qh+u).T   blocks hp
            qvT = pp.tile([P, 4 * S], F16)       # (qh+v).T
            khT = pp.tile([P, 4 * T], F16)
            rh2T = pp.tile([P, 4 * 3072], F16)
            vh16 = pp.tile([P, 16 * (HC * 65)], F16)  # per key tile: 8 h x (64+1)
            concatT = pp.tile([P, 4 * S], F16)
            WoS = pp.tile([P, 4 * D], F16)

            ub_sb = cp.tile([P, 4], F32)
            vb_sb = cp.tile([P, 4], F32)
            atril2_sb = cp.tile([P, P], mybir.dt.uint8)
            zeros_sb = cp.tile([P, P], F16)
            biasn_sb = cp.tile([P, 1], F32)
            ones_sb = cp.tile([P, DK], F32)
            ident_sb = cp.tile([P, P], F16)
            nc.vector.memset(biasn_sb[:], EXP_BIAS)
            nc.vector.memset(zeros_sb[:], 0.0)
            nc.vector.memset(ones_sb[:], 1.0)

            nc.scalar.dma_start(out=ub_sb[:], in_=ub[:])
            nc.scalar.dma_start(out=vb_sb[:], in_=vb[:])
            nc.scalar.dma_start(out=atril2_sb[:], in_=atril2[:])
            nc.scalar.dma_start(out=ident_sb[:], in_=ident[:])
            # WoS layout [128, dt*1024 + o] <- Wo16[(dt p), o], one DMA
            nc.scalar.dma_start(
                out=WoS[:],
                in_=bass.AP(Wo16, 0, [[D, P], [P * D, 4], [1, D]]),
            )

            # ---------------- projections + attention ----------------
            # attention-side pools are opened first so hp0's M stage can be
            # software-pipelined into the k/v projection chains; psm doubles
            # as the projection chain psum pool (PSUM budget: 2+4+2 = 8)
            with (
                tc.tile_pool(name="att_m", bufs=3) as mp,
                tc.tile_pool(name="dram", bufs=10, space="DRAM") as dp,
                tc.tile_pool(name="ps_m", bufs=2, space="PSUM") as psm,
                tc.tile_pool(name="ps_ac", bufs=4, space="PSUM") as psac,
                tc.tile_pool(name="ps_av", bufs=2, space="PSUM") as psav,
            ):
                mregs = {}

                def m_units_for(hp, split_evac):
                    """hp's M stage as a list of small closures, so it can be
                    software-pipelined into other work's emission.
                    One DRAM region per (h, Q) holding that quad's 4 slots."""
                    mregs[hp] = {(h, Q): dp.tile([QREG[Q]], F16, tag="mreg",
                                                 name="mreg")
                                 for h in range(2) for Q in range(2)}
                    units = []
                    msbs_state = {}

                    for qi in range(8):
                        Qd = qi // 4
                        Wq_ = w_of(qi)
                        nwc = cdiv(Wq_, 512)

                        def mk_wc(qi=qi, Qd=Qd, Wq_=Wq_, wc_=None):
                            def f(wc=wc_):
                                if qi not in msbs_state:
                                    msbs_state[qi] = [
                                        mp.tile([P, 2176], F16, tag="msb",
                                                name="msb") for _ in range(2)]
                                nw = min(512, Wq_ - wc * 512)
                                for h in range(2):
                                    pr = slice(h * DK, (h + 1) * DK)
                                    mps = psm.tile([P, 512], F32, tag="mps",
                                                   name="mps")
                                    nc.tensor.matmul(
                                        mps[:, :nw],
                                        qvT[pr, hp * S + qi * P :
                                            hp * S + (qi + 1) * P],
                                        rh2T[pr, hp * 3072 + qi * P + wc * 512 :
                                             hp * 3072 + qi * P + wc * 512 + nw],
                                        start=True, stop=True,
                                    )
                                    dst = msbs_state[qi][h][
                                        :, wc * 512 : wc * 512 + nw]
                                    if split_evac and (qi + h) % 2 == 0:
                                        nc.scalar.activation(
                                            dst, mps[:, :nw], AF.Copy)
                                    else:
                                        nc.vector.tensor_copy(dst, mps[:, :nw])
                            return f

                        for wc in range(nwc):
                            units.append(mk_wc(wc_=wc))

                        def mk_wr(qi=qi, Qd=Qd, Wq_=Wq_, hp=hp):
                            def f():
                                msbs = msbs_state.pop(qi)
                                base = SLOT_OFF[Qd][qi % 4]
                                for h in range(2):
                                    reg = mregs[hp][(h, Qd)]
                                    nc.gpsimd.dma_start(
                                        out=bass.AP(
                                            reg.tensor, reg.offset + base,
                                            [[Wq_, P], [1, Wq_]]),
                                        in_=msbs[h][:, :Wq_],
                                    )
                            return f

                        units.append(mk_wr())
                    return units

                def load_w(pool, wparam):
                    wsb = pool.tile([P, 8 * DH], BF16, tag="wsb")
                    nc.sync.dma_start(
                        out=wsb[:],
                        in_=bass.AP(wparam, 0, [[DH, P], [P * DH, 8], [1, DH]]),
                    )
                    return wsb

                def load_x(pool, xparam, th):
                    # 8 DMAs alternating the two HWDGE rings so the first
                    # chain can start after one chunk lands
                    xsb = pool.tile([P, 8 * 1024], BF16, tag="xstage")
                    L = xparam.shape[1]
                    for kd in range(8):
                        eng = nc.scalar if kd % 2 else nc.sync
                        eng.dma_start(
                            out=xsb[:, kd * 1024 : (kd + 1) * 1024],
                            in_=bass.AP(xparam, kd * P * L + th * 1024,
                                        [[L, P], [1, 1024]]),
                        )
                    return xsb

                def proj_T(pool, wsb, xparam, ncols, evac, pull=None):
                    # chain-major: 8 back-to-back matmuls per psum chain
                    nth = ncols // 1024
                    for th in range(nth):
                        xsb = load_x(pool, xparam, th)
                        for dot in range(4):
                            for tc2 in range(2):
                                ps = psm.tile([P, 512], F32, tag="mps",
                                              name="projps")
                                for kd in range(8):
                                    nc.tensor.matmul(
                                        ps[:],
                                        wsb[:, kd * DH + dot * P : kd * DH + (dot + 1) * P],
                                        xsb[:, kd * 1024 + tc2 * 512 : kd * 1024 + (tc2 + 1) * 512],
                                        start=(kd == 0),
                                        stop=(kd == 7),
                                    )
                                evac(ps, dot, th * 1024 + tc2 * 512)
                                if pull is not None:
                                    pull()
                with (
                    tc.tile_pool(name="projp", bufs=2) as jp,
                    tc.tile_pool(name="projw", bufs=2) as jw,
                    tc.tile_pool(name="rhtmp", bufs=1) as jr,
                ):
                    wsb = load_w(jw, Wq)

                    def evac_q(ps, dot, col):
                        nc.vector.tensor_scalar(
                            quT[:, dot * S + col : dot * S + col + 512], ps[:],
                            ub_sb[:, dot : dot + 1], None, OP.add)
                        nc.vector.tensor_scalar(
                            qvT[:, dot * S + col : dot * S + col + 512], ps[:],
                            vb_sb[:, dot : dot + 1], None, OP.add)

                    proj_T(jp, wsb, qT, S, evac_q)

                    rhT = jr.tile([P, 4 * T], F16, tag="rhT")
                    wsb = load_w(jw, Wr)

                    def evac_r(ps, dot, col):
                        nc.vector.tensor_copy(
                            rhT[:, dot * T + col : dot * T + col + 512], ps[:])

                    proj_T(jp, wsb, RT, T, evac_r)

                    # rh2T[:, m'] = rhT[:, (m' + 1023) % 2048], m' in [0, 3072)
                    for dot in range(4):
                        nc.vector.tensor_copy(
                            rh2T[:, dot * 3072 : dot * 3072 + 1025],
                            rhT[:, dot * T + 1023 : dot * T + 2048])
                        nc.vector.tensor_copy(
                            rh2T[:, dot * 3072 + 1025 : dot * 3072 + 3072],
                            rhT[:, dot * T : dot * T + 2047])

                    # k and v projections, with hp0's M stage pipelined in
                    units0 = m_units_for(0, split_evac=True)
                    pstate = {"i": 0, "chain": 0}
                    nchains = (T // 1024) * 8 + 2 * 8  # k chains + v chains

                    def pull():
                        pstate["chain"] += 1
                        tgt = min(len(units0),
                                  pstate["chain"] * len(units0) // nchains)
                        while pstate["i"] < tgt:
                            units0[pstate["i"]]()
                            pstate["i"] += 1

                    wsb = load_w(jw, Wk)

                    def evac_k(ps, dot, col):
                        nc.vector.tensor_copy(
                            khT[:, dot * T + col : dot * T + col + 512], ps[:])

                    proj_T(jp, wsb, kT, T, evac_k, pull=pull)

                    # vh (untransposed): per key tile tt, psum [128 keys, 512 dh]
                    wsb = load_w(jw, Wv)
                    for tg in range(2):
                        vsb = load_x(jp, vT, tg)
                        for tl in range(8):
                            ps = psm.tile([P, 512], F32, tag="mps", name="vhps")
                            for kd in range(8):
                                nc.tensor.matmul(
                                    ps[:],
                                    vsb[:, kd * 1024 + tl * P : kd * 1024 + (tl + 1) * P],
                                    wsb[:, kd * DH : (kd + 1) * DH],
                                    start=(kd == 0),
                                    stop=(kd == 7),
                                )
                            tt = tg * 8 + tl
                            base = tt * (HC * 65)
                            dst = bass.AP(vh16.tensor, vh16.offset + base,
                                          [[vh16.tensor.shape[1], P], [65, HC], [1, DK]])
                            nc.vector.tensor_copy(
                                dst, ps[:].rearrange("p (h c) -> p h c", h=HC))
                            ones = bass.AP(vh16.tensor, vh16.offset + base + DK,
                                           [[vh16.tensor.shape[1], P], [65, HC]])
                            nc.vector.memset(ones, 1.0)
                            pull()
                    while pstate["i"] < len(units0):
                        units0[pstate["i"]]()
                        pstate["i"] += 1

                # ---------------- attention quad stages ----------------
                with (
                    tc.tile_pool(name="att_att", bufs=6) as atp,
                    tc.tile_pool(name="att_ebd", bufs=18) as bp,
                    tc.tile_pool(name="nrm", bufs=2) as np_,
                    tc.tile_pool(name="nrmb", bufs=2) as npb,
                ):
                    g_ebds = {}

                    def emit_read_g(hp_, Q, qi):
                        # plain sheared read: bd[il, j] = M[il, il+j], rows at
                        # stride W+1 against the stride-W write
                        Wq_ = w_of(qi)
                        KQ = kq_of(qi)
                        base = SLOT_OFF[Q][qi - 4 * Q]
                        for h in range(2):
                            ebd = bp.tile([P, 2048], F16, tag="ebd",
                                          name="ebd")
                            reg = mregs[hp_][(h, Q)]
                            eng = nc.scalar if h else nc.sync
                            eng.dma_start(
                                out=ebd[:, :KQ],
                                in_=bass.AP(
                                    reg.tensor, reg.offset + base,
                                    [[Wq_ + 1, P], [1, KQ]]))
                            g_ebds[(hp_, Q, h, qi)] = ebd

                    def quad_stage(hp, nxt_units):
                        mreg = mregs[hp]
                        upos = 0
                        bi = 0
                        for Q in range(2):
                            # dense burst to re-arm the HAM clock gate
                            kal = psac.tile([P, 512], F32, tag="acps",
                                            name="kal")
                            for _ in range(8):
                                nc.tensor.matmul(
                                    kal[:], ident_sb[:], khT[0:P, 0:512],
                                    start=True, stop=True)

                            avps = [psav.tile([P, 512], F32, tag="avps",
                                              name="avps") for _ in range(2)]
                            nkb = 12 if Q == 0 else 16
                            acs = {}

                            def emit_ac(kb, Q=Q, acs=acs, hp=hp):
                                for h in range(2):
                                    pr = slice(h * DK, (h + 1) * DK)
                                    qs = max(4 * Q, kb - 8)
                                    n_q = (4 * Q + 4 - qs) * P
                                    acps = psac.tile([P, 512], F32,
                                                     tag="acps", name="acps")
                                    nc.tensor.matmul(
                                        acps[:, :n_q],
                                        khT[pr, hp * T + kb * P :
                                            hp * T + (kb + 1) * P],
                                        quT[pr, hp * S + qs * P :
                                            hp * S + qs * P + n_q],
                                        start=True, stop=False,
                                        skip_group_check=True,
                                    )
                                    acs[(h, kb)] = (acps, qs, n_q)

                            def emit_add(kb, Q=Q, acs=acs, hp=hp):
                                # acps[k, q] += bd^T: bd tile as the
                                # stationary operand, identity moving
                                for h in range(2):
                                    acps, qs, n_q = acs[(h, kb)]
                                    for qi in range(qs, 4 * Q + 4):
                                        qloc = (qi - qs) * P
                                        nc.tensor.matmul(
                                            acps[:, qloc : qloc + P],
                                            g_ebds[(hp, Q, h, qi)][
                                                :, kb * P : (kb + 1) * P],
                                            ident_sb[:],
                                            start=False,
                                            stop=(qi == 4 * Q + 3),
                                            skip_group_check=True,
                                        )

                            atts = {}

                            def emit_exp(kb, Q=Q, acs=acs, atts=atts):
                                for h in range(2):
                                    acps, qs, n_q = acs.pop((h, kb))
                                    att = atp.tile([P, 512], F16, tag="att",
                                                   name="att")
                                    nc.scalar.activation(
                                        att[:, :n_q], acps[:, :n_q], AF.Exp,
                                        bias=biasn_sb[:], scale=0.125)
                                    if kb >= 4 * Q + 8:
                                        nc.vector.copy_predicated(
                                            att[:, 0:P], atril2_sb[:],
                                            zeros_sb[:])
                                    atts[(h, kb)] = (att, qs, n_q)

                            def emit_avmm(kb, last, Q=Q, atts=atts,
                                          avps=avps, hp=hp):
                                for h in range(2):
                                    att, qs, n_q = atts.pop((h, kb))
                                    qloc = (qs - 4 * Q) * P
                                    col = kb * (HC * 65) + (hp * 2 + h) * 65
                                    nc.tensor.matmul(
                                        avps[h][0:65, qloc : qloc + n_q],
                                        vh16[:, col : col + 65],
                                        att[:, :n_q],
                                        start=(kb == 0), stop=last,
                                        skip_group_check=True,
                                    )

                            for qi in range(4 * Q, 4 * Q + 4):
                                if (hp, Q, 0, qi) not in g_ebds:
                                    emit_read_g(hp, Q, qi)
                            emit_ac(0)
                            emit_add(0)
                            for i in range(nkb):
                                bi += 1
                                emit_exp(i)
                                if Q == 0 and i == 6:
                                    for qi in range(4, 8):
                                        emit_read_g(hp, 1, qi)
                                if Q == 1 and i == 8 and hp < 3:
                                    for qi in range(0, 4):
                                        emit_read_g(hp + 1, 0, qi)
                                if i + 1 < nkb:
                                    emit_ac(i + 1)
                                # finish the next hp's M stage by ~beat 20
                                # so its writes land well before the reads
                                target = bi * len(nxt_units) // 24
                                while upos < min(target, len(nxt_units)):
                                    nxt_units[upos]()
                                    upos += 1
                                emit_avmm(i, last=(i == nkb - 1))
                                if i + 1 < nkb:
                                    emit_add(i + 1)
                            for qi in range(4 * Q, 4 * Q + 4):
                                del g_ebds[(hp, Q, 0, qi)]
                                del g_ebds[(hp, Q, 1, qi)]

                            # seam drain: PE work with no new deps, emitted
                            # before the norm's recip-gated matmuls
                            seam_target = ((bi + 8) * len(nxt_units) // 24
                                           if Q == 0 else len(nxt_units))
                            while upos < min(seam_target, len(nxt_units)):
                                nxt_units[upos]()
                                upos += 1

                            # ---- normalization for this quad ----
                            dcol = np_.tile([P, 512], F32, tag="dcol",
                                            name="dcol")
                            rcol = np_.tile([P, 512], F32, tag="rcol",
                                            name="rcol")
                            nc.vector.memset(dcol[:], 1.0)
                            for h in range(2):
                                c = 32 * h
                                nc.scalar.activation(
                                    dcol[c : c + 1, :],
                                    avps[h][64:65, :], AF.Copy)
                            nc.vector.reciprocal(rcol[:], dcol[:])
                            for h in range(2):
                                c = 32 * h
                                rbp = psac.tile([P, 512], F32, tag="acps",
                                                name="rbp")
                                nc.tensor.matmul(
                                    rbp[0:DK, :], ones_sb[c : c + 1, :],
                                    rcol[c : c + 1, :], start=True, stop=True,
                                    tile_position=(c, 0))
                                rbc = npb.tile([DK, 512], F32, tag="rbc")
                                nc.vector.tensor_copy(rbc[:], rbp[0:DK, :])
                                nc.vector.tensor_tensor(
                                    concatT[h * DK : (h + 1) * DK,
                                            hp * S + Q * 512 :
                                            hp * S + (Q + 1) * 512],
                                    avps[h][0:DK, :], rbc[:], OP.mult)
                        while upos < len(nxt_units):
                            nxt_units[upos]()
                            upos += 1

                    for hp in range(4):
                        nxt = (m_units_for(hp + 1, split_evac=False)
                               if hp < 3 else [])
                        quad_stage(hp, nxt)

            # ---------------- output projection ----------------
            with (
                tc.tile_pool(name="outp", bufs=2) as op_,
                tc.tile_pool(name="outpsum", bufs=4, space="PSUM") as ops_,
            ):
                for it in range(8):
                    osb = op_.tile([P, 1024], F32, tag="osb")
                    for oc in range(2):
                        ps = ops_.tile([P, 512], F32, tag="out")
                        for dt in range(4):
                            nc.tensor.matmul(
                                ps[:],
                                concatT[:, dt * S + it * P : dt * S + (it + 1) * P],
                                WoS[:, dt * D + oc * 512 : dt * D + (oc + 1) * 512],
                                start=(dt == 0), stop=(dt == 3),
                            )
                        nc.vector.tensor_copy(osb[:, oc * 512 : (oc + 1) * 512], ps[:])
                    nc.gpsimd.dma_start(
                        out=outp[it * P : (it + 1) * P, :], in_=osb[:])

    if split_waits:
        _split_multi_waits(nc)
    return nc


def prep_core_inputs(core, q, k, v, u, v_bias, Wq, Wk, Wv, Wr, Wo, R):
    b, hh = core // 2, core % 2
    sl = slice(hh * DH, (hh + 1) * DH)
    BF = ml_dtypes.bfloat16
    return {
        "qT": q[b].T.astype(BF),
        "kT": k[b].T.astype(BF),
        "vT": v[b].T.astype(BF),
        "RT": R.T.astype(BF),
        "Wq": Wq[sl, :].T.astype(BF),
        "Wk": Wk[sl, :].T.astype(BF),
        "Wv": Wv[sl, :].T.astype(BF),
        "Wr": Wr[sl, :].T.astype(BF),
        "Wo16": Wo[:, sl].T.astype(np.float16),
        "ub": np.ascontiguousarray(
            u[0, hh * HC : (hh + 1) * HC, 0, :].reshape(4, P).T),
        "vb": np.ascontiguousarray(
            v_bias[0, hh * HC : (hh + 1) * HC, 0, :].reshape(4, P).T),
        "atril2": np.tril(np.ones((P, P), np.uint8), -1),
        "ident": np.eye(P, dtype=np.float16),
    }


def combine_outputs(results):
    # results: list of 8 dicts with "out" [S, D]; partial sums per batch pair
    out = np.empty((4, S, D), np.float32)
    for b in range(4):
        out[b] = results[2 * b]["out"] + results[2 * b + 1]["out"]
    return out


_CACHED_NC = None
last_result = None  # BassKernelResults of the most recent run (for test harness)


def kernel(q, k, v, mask, u, v_bias, Wq, Wk, Wv, Wr, Wo, R):
    global _CACHED_NC, last_result
    from concourse.bass_utils import run_bass_kernel_spmd

    q, k, v = np.asarray(q), np.asarray(k), np.asarray(v)
    u, v_bias = np.asarray(u), np.asarray(v_bias)
    Wq, Wk, Wv, Wr, Wo, R = map(np.asarray, (Wq, Wk, Wv, Wr, Wo, R))

    # The kernel exploits the known TXL mask structure (j <= i + MEM).
    # Verify the passed mask matches; structural masking is baked in.
    m = np.asarray(mask)
    exp_mask = (np.arange(T)[None, :] <= np.arange(S)[:, None] + 1024)
    assert m.shape == (4, S, T) and bool((m == exp_mask[None]).all()), \
        "kernel compiled for the TXL causal mask (j <= i + MEM)"

    if _CACHED_NC is None:
        _CACHED_NC = build_nc()

    in_maps = [prep_core_inputs(c, q, k, v, u, v_bias, Wq, Wk, Wv, Wr, Wo, R)
               for c in range(8)]
    trace = bool(os.environ.get("TXL_TRACE"))
    last_result = run_bass_kernel_spmd(
        _CACHED_NC, in_maps, list(range(8)), trace=trace,
        trace_cores=[0] if trace else None)
    return combine_outputs(last_result.results)
